# revision 23
# baseline (speedup 1.0000x reference)
"""CQVAE loss kernel for Trainium2, data-parallel over batch on 8 NeuronCores.

loss = kld(qy) + mse(gather(rzs), zs[:, :Sg]) + bias(best, best_gt)
       + bias(gather(pts), gts)
where bias(p, g) = mse(p, g) + 10 * mse(p[..., MARK, :], g[..., MARK, :]).

Each core handles 16 of the 128 batches.  The mapping-gathers run as
dma_gather ops (hundreds of rows per op, ~9ns/row of Q7 emission)
interleaved so gather bytes, zs bytes and compute pipeline smoothly.
pts/gts rows are zero-padded to 256 floats on the host so gathered rows
are 1KB-aligned and pad columns contribute nothing to the sums.  zs/gts
are laid out so every partition reads one contiguous 64/16KB run.  Each
core ships a [128, 32] per-partition stats tile; the host folds
partitions and cores.
"""

import sys

import numpy as np

try:
    import concourse  # noqa: F401
except ImportError:  # pragma: no cover
    sys.path.insert(0, "/opt/trn_rl_repo")

import ml_dtypes

import concourse.bass as bass  # noqa: F401
import concourse.mybir as mybir
import concourse.tile as tile
from concourse import bacc
from concourse.bass_utils import run_bass_kernel_spmd

F32 = mybir.dt.float32
F8 = mybir.dt.float8e4
BF16 = mybir.dt.bfloat16
I32 = mybir.dt.int32
AX = mybir.AxisListType
OP = mybir.AluOpType
ACTF = mybir.ActivationFunctionType

NCORES = 8
B, S, SG, D, P, V = 128, 256, 128, 1024, 118, 64
BL = B // NCORES  # batches per core
P2 = 2 * P  # 236 true floats per point-row
PC = 256  # padded point-row width
MARK = (0, 29, 88, 117)
EPS = 1e-20
ALPHA = 10.0
MW = float(np.sqrt(1.0 + ALPHA * P2 / (2 * len(MARK))))  # 17.2047 best-mark fold

NSTAT = 36
# stats columns
C_KLD = 33
C_BEST, C_BESTM = 10, 11
C_AE = 0  # 10 cols: ae pieces
C_BIAS = 12  # 4 cols: bias sq totals per pts quarter
C_MARK = 16  # 16 cols: 4 marks x 4 quarters

CW = D + PC  # 1280 combined row width

# rzs pieces by (start_slot, n_slots): coarse early, 1-slot at the end
AE_PIECES = [(0, 2), (2, 2), (4, 2), (6, 2), (8, 2), (10, 2),
             (12, 1), (13, 1), (14, 1), (15, 1)]
NAE = len(AE_PIECES)
NPT = 4  # pts gather ops / gts quarters (4 batches each)
KP = BL // NPT  # 4 batch-slots per pts quarter

_module = None
last_results = None  # BassKernelResults of the most recent run (for profiling)


def _build_module():
    nc = bacc.Bacc()

    zs = nc.dram_tensor("zs", [BL * SG, D], F8, kind="ExternalInput")
    # comb row r = concat(rzs[r], pts_padded[r]) — one gather fetches both
    comb = nc.dram_tensor("comb", [BL * S, CW], BF16, kind="ExternalInput")
    gts = nc.dram_tensor("gts", [BL * SG, PC], BF16, kind="ExternalInput")
    qy = nc.dram_tensor("qy", [BL * S, V], BF16, kind="ExternalInput")
    best = nc.dram_tensor("best", [BL, P2], F32, kind="ExternalInput")
    best_gt = nc.dram_tensor("best_gt", [BL, P2], F32, kind="ExternalInput")
    # idx[p, k] = (p//8)*S + mapping[p//8, 16*(p%8) + k] — the flat source
    # row for slot k of partition p, shared by the rzs and pts gathers
    idx2 = nc.dram_tensor("idx2", [128, BL], I32, kind="ExternalInput")
    out = nc.dram_tensor("out", [128, NSTAT], F32, kind="ExternalOutput")

    QCOLS = BL * S * V // 128  # 2048
    QN = BL * S // 128  # 32 qy rows per partition

    with tile.TileContext(nc) as tc:
        with tc.tile_pool(name="cst", bufs=1) as cst:
            idx_t = cst.tile([128, BL], I32)
            nc.sync.dma_start(idx_t[:], idx2[:])

            stats = cst.tile([128, NSTAT], F32)
            nc.vector.memset(stats[:], 0.0)

            # ---- gathers: one combined-row op per slot (SWDGE queue) ------
            cb = cst.tile([128, BL * CW], BF16)
            for k in range(BL):
                nc.gpsimd.indirect_dma_start(
                    out=cb[:, k * CW : (k + 1) * CW],
                    out_offset=None,
                    in_=comb[:],
                    in_offset=bass.IndirectOffsetOnAxis(
                        ap=idx_t[:, k : k + 1], axis=0
                    ),
                )
            cb3 = cb[:].rearrange("p (k c) -> p k c", c=CW)

            # ---- direct loads --------------------------------------------
            # all on the sync HWDGE queue: the scalar queue's dma_start ops
            # cost the Act sequencer ~0.8us each, and Act is the tail-maker
            qy_t = cst.tile([128, QCOLS], BF16)
            nc.sync.dma_start(
                qy_t[:], qy[:].rearrange("(p n) v -> p (n v)", n=QN)
            )
            bt = cst.tile([BL, P2], F32)
            nc.scalar.dma_start(bt[:], best[:])
            bgt = cst.tile([BL, P2], F32)
            nc.scalar.dma_start(bgt[:], best_gt[:])
            # partition p holds gts rows 16p..16p+15 (contiguous 16KB)
            gts_r = gts[:].rearrange("(p k) c -> p (k c)", k=BL)
            gt_h = []
            for h in range(NPT):
                g = cst.tile([128, KP * PC], BF16, tag=f"gt{h}", name=f"gt{h}")
                nc.scalar.dma_start(g[:], gts_r[:, h * KP * PC : (h + 1) * PC * KP])
                gt_h.append(g)

            # sync HWDGE queue: zs pieces (8.4 MB)
            # partition p holds zs rows 16p..16p+15 (contiguous 64KB)
            zs_r = zs[:].rearrange("(p k) d -> p (k d)", k=BL)
            zs_t = []
            for j, (s0, ns) in enumerate(AE_PIECES):
                z = cst.tile([128, ns * D], F8, tag=f"zs{j}", name=f"zs{j}")
                nc.sync.dma_start(z[:], zs_r[:, s0 * D : (s0 + ns) * D])
                zs_t.append(z)

            # ---- compute --------------------------------------------------
            # BEST (tiny; the 10x landmark weights are folded into host-side
            # column scales, so one squared-sum accumulator suffices)
            nc.vector.tensor_sub(bt[:], bt[:], bgt[:])
            nc.vector.scalar_tensor_tensor(
                out=bgt[:], in0=bt[:], scalar=0.0, in1=bt[:],
                op0=OP.subtract, op1=OP.mult,
                accum_out=stats[:BL, C_BEST : C_BEST + 1],
            )

            # KLD: sum q * (log(q + eps) - log(1/V)) via log(V*q + V*eps)
            lg = cst.tile([128, QCOLS], F32)
            ebias = cst.tile([128, 1], F32)
            nc.vector.memset(ebias[:], float(V) * EPS)
            nc.scalar.activation(lg[:], qy_t[:], ACTF.Ln, bias=ebias[:], scale=float(V))
            nc.vector.scalar_tensor_tensor(
                out=lg[:],
                in0=lg[:],
                scalar=0.0,
                in1=qy_t[:],
                op0=OP.subtract,
                op1=OP.mult,
                accum_out=stats[:, C_KLD : C_KLD + 1],
            )

            zscr = cst.tile([128, 2 * D], BF16)

            def ae_piece(j, on_dve=False):
                s0, ns = AE_PIECES[j]
                rg = cb3[:, s0 : s0 + ns, 0:D]
                z3 = zs_t[j][:].rearrange("p (k d) -> p k d", d=D)
                nc.vector.tensor_sub(rg, rg, z3)
                acc = stats[:, C_AE + j : C_AE + j + 1]
                if on_dve:
                    # square-and-accumulate on DVE into a bf16 scratch
                    sout = zscr[:, : ns * D].rearrange("p (k d) -> p k d", d=D)
                    nc.vector.scalar_tensor_tensor(
                        out=sout, in0=rg, scalar=0.0, in1=rg,
                        op0=OP.subtract, op1=OP.mult, accum_out=acc,
                    )
                else:
                    nc.scalar.activation(rg, rg, ACTF.Square, accum_out=acc)

            def bias_quarter(h, on_dve=False):
                pg = cb3[:, h * KP : (h + 1) * KP, D : D + PC]
                g3 = gt_h[h][:].rearrange("p (k c) -> p k c", c=PC)
                nc.vector.tensor_sub(pg, pg, g3)
                acc = stats[:, C_BIAS + h : C_BIAS + h + 1]
                if on_dve:
                    # squares land in the consumed gts tile; marks read there
                    nc.vector.scalar_tensor_tensor(
                        out=g3, in0=pg, scalar=0.0, in1=pg,
                        op0=OP.subtract, op1=OP.mult, accum_out=acc,
                    )
                    sq = g3
                else:
                    nc.scalar.activation(pg, pg, ACTF.Square, accum_out=acc)
                    sq = pg
                # pts cols are host-permuted so the 8 mark cols lead: one
                # contiguous reduce per quarter instead of 4 strided ones
                cm = C_MARK + h
                nc.vector.reduce_sum(
                    out=stats[:, cm : cm + 1], in_=sq[:, :, 0:8], axis=AX.XY
                )

            # compute in data-arrival order
            ae_piece(0)
            ae_piece(1)
            bias_quarter(0)
            ae_piece(2)
            bias_quarter(1)
            ae_piece(3)
            bias_quarter(2)
            ae_piece(4)
            bias_quarter(3, on_dve=True)
            ae_piece(5, on_dve=True)
            ae_piece(6, on_dve=True)
            ae_piece(7)
            ae_piece(8, on_dve=True)
            ae_piece(9)

            nc.sync.dma_start(out[:], stats[:])

    nc.compile()
    return nc


def kernel(
    zs, rzs, pts, best, qy, gts, best_gt, mapping, vector_dims, **trace_kwargs
):
    global _module, last_results
    vd = int(np.asarray(vector_dims))
    assert vd == V, f"kernel compiled for vector_dims={V}, got {vd}"

    if _module is None:
        _module = _build_module()

    BF = ml_dtypes.bfloat16
    zs = np.asarray(zs, dtype=np.float32)
    qy = np.asarray(qy, dtype=np.float32).astype(BF)
    mapping = np.asarray(mapping).astype(np.int32)

    # best: fold the 10x landmark mse into column scales (f32, no overflow)
    wcol = np.ones(P2, np.float32)
    wcol[2 * np.array(MARK)] = MW
    wcol[2 * np.array(MARK) + 1] = MW
    best2 = np.ascontiguousarray(
        np.asarray(best, dtype=np.float32).reshape(B, P2) * wcol
    )
    bgt2 = np.ascontiguousarray(
        np.asarray(best_gt, dtype=np.float32).reshape(B, P2) * wcol
    )

    # point-column permutation: the 8 mark columns first, so the device can
    # take each quarter's mark sum with one contiguous reduce
    rest = [i for i in range(P) if i not in MARK]
    perm = np.array(list(MARK) + rest)

    # combined gather rows: [rzs | pts(perm) zero-padded to PC], bf16
    comb = np.zeros((B, S, CW), dtype=BF)
    comb[:, :, :D] = np.asarray(rzs, dtype=np.float32).reshape(B, S, D).astype(BF)
    comb[:, :, D : D + P2] = (
        np.asarray(pts, dtype=np.float32)[:, :, perm, :].reshape(B, S, P2).astype(BF)
    )
    gts_p = np.zeros((B, SG, PC), dtype=BF)
    gts_p[:, :, :P2] = (
        np.asarray(gts, dtype=np.float32)[:, :, perm, :].reshape(B, SG, P2).astype(BF)
    )

    pp = np.arange(128)
    in_maps = []
    for c in range(NCORES):
        sl = slice(c * BL, (c + 1) * BL)
        mp = mapping[sl]  # [BL, SG]
        b = pp // 8
        pos = 16 * (pp % 8)[:, None] + np.arange(BL)[None, :]
        idx2 = (b[:, None] * S + mp[b[:, None], pos]).astype(np.int32)
        # zs rows reordered so partition p holds rows 16p..16p+15:
        # row 16p+k = zs[b, 16q+k] -> natural order already (b-major, i-minor)
        in_maps.append(
            {
                "zs": np.ascontiguousarray(zs[sl, :SG].reshape(BL * SG, D).astype(ml_dtypes.float8_e4m3)),
                "comb": comb[sl].reshape(BL * S, CW),
                "gts": gts_p[sl].reshape(BL * SG, PC),
                "qy": qy[sl].reshape(BL * S, V),
                "best": np.ascontiguousarray(best2[sl]),
                "best_gt": np.ascontiguousarray(bgt2[sl]),
                "idx2": np.ascontiguousarray(idx2),
            }
        )

    last_results = run_bass_kernel_spmd(
        _module, in_maps, list(range(NCORES)), **trace_kwargs
    )
    parts = np.stack(
        [
            np.asarray(r["out"], dtype=np.float64).reshape(128, NSTAT).sum(axis=0)
            for r in last_results.results
        ]
    )
    tot = parts.sum(axis=0)

    ae_loss = tot[C_AE : C_AE + NAE].sum() / (B * SG * D)
    bias_sq = tot[C_BIAS : C_BIAS + NPT].sum()
    mark_sq = tot[C_MARK : C_MARK + NPT].sum()
    bias_loss = bias_sq / (B * SG * P2) + ALPHA * mark_sq / (B * SG * 2 * len(MARK))
    kld_loss = tot[C_KLD] / (B * S)
    best_mse = tot[C_BEST] / (B * P2)

    return np.array(kld_loss + ae_loss + best_mse + bias_loss, dtype=np.float32)



# revision 25
# speedup vs baseline: 1.0601x; 1.0601x over previous
"""CQVAE loss kernel for Trainium2, data-parallel over batch on 8 NeuronCores.

loss = kld(qy) + mse(gather(rzs), zs[:, :Sg]) + bias(best, best_gt)
       + bias(gather(pts), gts)
where bias(p, g) = mse(p, g) + 10 * mse(p[..., MARK, :], g[..., MARK, :]).

Each core handles 16 of the 128 batches.  The mapping-gathers run as
dma_gather ops (hundreds of rows per op, ~9ns/row of Q7 emission)
interleaved so gather bytes, zs bytes and compute pipeline smoothly.
pts/gts rows are zero-padded to 256 floats on the host so gathered rows
are 1KB-aligned and pad columns contribute nothing to the sums.  zs/gts
are laid out so every partition reads one contiguous 64/16KB run.  Each
core ships a [128, 32] per-partition stats tile; the host folds
partitions and cores.
"""

import sys

import numpy as np

try:
    import concourse  # noqa: F401
except ImportError:  # pragma: no cover
    sys.path.insert(0, "/opt/trn_rl_repo")

import ml_dtypes

import concourse.bass as bass  # noqa: F401
import concourse.mybir as mybir
import concourse.tile as tile
from concourse import bacc
from concourse.bass_utils import run_bass_kernel_spmd

F32 = mybir.dt.float32
BF16 = mybir.dt.bfloat16
I32 = mybir.dt.int32
AX = mybir.AxisListType
OP = mybir.AluOpType
ACTF = mybir.ActivationFunctionType

NCORES = 8
B, S, SG, D, P, V = 128, 256, 128, 1024, 118, 64
BL = B // NCORES  # batches per core
P2 = 2 * P  # 236 true floats per point-row
PC = 256  # padded point-row width
MARK = (0, 29, 88, 117)
EPS = 1e-20
ALPHA = 10.0
MW = float(np.sqrt(1.0 + ALPHA * P2 / (2 * len(MARK))))  # 17.2047 best-mark fold

NSTAT = 36
# stats columns
C_KLD = 33
C_BEST, C_BESTM = 10, 11
C_AE = 0  # 10 cols: ae pieces
C_BIAS = 12  # 4 cols: bias sq totals per pts quarter
C_MARK = 16  # 16 cols: 4 marks x 4 quarters

CW = D + PC  # 1280 combined row width

# rzs pieces by (start_slot, n_slots): coarse early, 1-slot at the end
AE_PIECES = [(0, 2), (2, 2), (4, 2), (6, 2), (8, 2), (10, 2),
             (12, 1), (13, 1), (14, 1), (15, 1)]
NAE = len(AE_PIECES)
NPT = 4  # pts gather ops / gts quarters (4 batches each)
KP = BL // NPT  # 4 batch-slots per pts quarter

_module = None
last_results = None  # BassKernelResults of the most recent run (for profiling)


def _build_module():
    nc = bacc.Bacc()

    zs = nc.dram_tensor("zs", [BL * SG, D], BF16, kind="ExternalInput")
    # comb row r = concat(rzs[r], pts_padded[r]) — one gather fetches both
    comb = nc.dram_tensor("comb", [BL * S, CW], BF16, kind="ExternalInput")
    gts = nc.dram_tensor("gts", [BL * SG, PC], BF16, kind="ExternalInput")
    qy = nc.dram_tensor("qy", [BL * S, V], BF16, kind="ExternalInput")
    best = nc.dram_tensor("best", [BL, P2], F32, kind="ExternalInput")
    best_gt = nc.dram_tensor("best_gt", [BL, P2], F32, kind="ExternalInput")
    # idx[p, k] = (p//8)*S + mapping[p//8, 16*(p%8) + k] — the flat source
    # row for slot k of partition p, shared by the rzs and pts gathers
    idx2 = nc.dram_tensor("idx2", [128, BL], I32, kind="ExternalInput")
    out = nc.dram_tensor("out", [128, NSTAT], F32, kind="ExternalOutput")

    QCOLS = BL * S * V // 128  # 2048
    QN = BL * S // 128  # 32 qy rows per partition

    with tile.TileContext(nc) as tc:
        with tc.tile_pool(name="cst", bufs=1) as cst:
            idx_t = cst.tile([128, BL], I32)
            nc.sync.dma_start(idx_t[:], idx2[:])

            stats = cst.tile([128, NSTAT], F32)
            nc.vector.memset(stats[:], 0.0)

            # ---- gathers: one combined-row op per slot (SWDGE queue) ------
            cb = cst.tile([128, BL * CW], BF16)
            for k in range(BL):
                nc.gpsimd.indirect_dma_start(
                    out=cb[:, k * CW : (k + 1) * CW],
                    out_offset=None,
                    in_=comb[:],
                    in_offset=bass.IndirectOffsetOnAxis(
                        ap=idx_t[:, k : k + 1], axis=0
                    ),
                )
            cb3 = cb[:].rearrange("p (k c) -> p k c", c=CW)

            # ---- direct loads --------------------------------------------
            # scalar HWDGE queue: qy, best, gts quarters (3.2 MB)
            qy_t = cst.tile([128, QCOLS], BF16)
            nc.scalar.dma_start(
                qy_t[:], qy[:].rearrange("(p n) v -> p (n v)", n=QN)
            )
            bt = cst.tile([BL, P2], F32)
            nc.sync.dma_start(bt[:], best[:])
            bgt = cst.tile([BL, P2], F32)
            nc.sync.dma_start(bgt[:], best_gt[:])
            # partition p holds gts rows 16p..16p+15 (contiguous 16KB)
            gts_r = gts[:].rearrange("(p k) c -> p (k c)", k=BL)
            gt_h = []
            for h in range(NPT):
                g = cst.tile([128, KP * PC], BF16, tag=f"gt{h}", name=f"gt{h}")
                nc.scalar.dma_start(g[:], gts_r[:, h * KP * PC : (h + 1) * KP * PC])
                gt_h.append(g)

            # sync HWDGE queue: zs pieces (8.4 MB)
            # partition p holds zs rows 16p..16p+15 (contiguous 64KB)
            zs_r = zs[:].rearrange("(p k) d -> p (k d)", k=BL)
            zs_t = []
            for j, (s0, ns) in enumerate(AE_PIECES):
                z = cst.tile([128, ns * D], BF16, tag=f"zs{j}", name=f"zs{j}")
                nc.sync.dma_start(z[:], zs_r[:, s0 * D : (s0 + ns) * D])
                zs_t.append(z)

            # ---- compute --------------------------------------------------
            # BEST (tiny, lands early on the scalar queue)
            nc.vector.tensor_sub(bt[:], bt[:], bgt[:])
            nc.vector.scalar_tensor_tensor(
                out=bgt[:], in0=bt[:], scalar=0.0, in1=bt[:],
                op0=OP.subtract, op1=OP.mult,
                accum_out=stats[:BL, C_BEST : C_BEST + 1],
            )

            # KLD: sum q * (log(q + eps) - log(1/V)) via log(V*q + V*eps)
            lg = cst.tile([128, QCOLS], F32)
            ebias = cst.tile([128, 1], F32)
            nc.vector.memset(ebias[:], float(V) * EPS)
            nc.scalar.activation(lg[:], qy_t[:], ACTF.Ln, bias=ebias[:], scale=float(V))
            nc.vector.scalar_tensor_tensor(
                out=lg[:],
                in0=lg[:],
                scalar=0.0,
                in1=qy_t[:],
                op0=OP.subtract,
                op1=OP.mult,
                accum_out=stats[:, C_KLD : C_KLD + 1],
            )

            def ae_piece(j, on_dve=False):
                s0, ns = AE_PIECES[j]
                rg = cb3[:, s0 : s0 + ns, 0:D]
                z3 = zs_t[j][:].rearrange("p (k d) -> p k d", d=D)
                nc.vector.tensor_sub(rg, rg, z3)
                acc = stats[:, C_AE + j : C_AE + j + 1]
                if on_dve:
                    # square-and-accumulate on DVE; the consumed zs tile is
                    # the scratch destination (no operand aliasing)
                    nc.vector.scalar_tensor_tensor(
                        out=z3, in0=rg, scalar=0.0, in1=rg,
                        op0=OP.subtract, op1=OP.mult, accum_out=acc,
                    )
                else:
                    nc.scalar.activation(rg, rg, ACTF.Square, accum_out=acc)

            def bias_quarter(h, on_dve=False):
                pg = cb3[:, h * KP : (h + 1) * KP, D : D + PC]
                g3 = gt_h[h][:].rearrange("p (k c) -> p k c", c=PC)
                nc.vector.tensor_sub(pg, pg, g3)
                acc = stats[:, C_BIAS + h : C_BIAS + h + 1]
                if on_dve:
                    # squares land in the consumed gts tile; marks read there
                    nc.vector.scalar_tensor_tensor(
                        out=g3, in0=pg, scalar=0.0, in1=pg,
                        op0=OP.subtract, op1=OP.mult, accum_out=acc,
                    )
                    sq = g3
                else:
                    nc.scalar.activation(pg, pg, ACTF.Square, accum_out=acc)
                    sq = pg
                cm = C_MARK + h
                nc.vector.reduce_sum(
                    out=stats[:, cm : cm + 1], in_=sq[:, :, 0:8], axis=AX.XY
                )

            # compute in data-arrival order
            ae_piece(0)
            ae_piece(1)
            bias_quarter(0)
            ae_piece(2)
            bias_quarter(1)
            ae_piece(3)
            bias_quarter(2)
            ae_piece(4)
            bias_quarter(3, on_dve=True)
            ae_piece(5, on_dve=True)
            ae_piece(6, on_dve=True)
            ae_piece(7)
            ae_piece(8, on_dve=True)
            ae_piece(9)

            nc.sync.dma_start(out[:], stats[:])

    nc.compile()
    return nc


def kernel(
    zs, rzs, pts, best, qy, gts, best_gt, mapping, vector_dims, **trace_kwargs
):
    global _module, last_results
    vd = int(np.asarray(vector_dims))
    assert vd == V, f"kernel compiled for vector_dims={V}, got {vd}"

    if _module is None:
        _module = _build_module()

    BF = ml_dtypes.bfloat16
    zs = np.asarray(zs, dtype=np.float32)
    qy = np.asarray(qy, dtype=np.float32).astype(BF)
    mapping = np.asarray(mapping).astype(np.int32)
    wcol = np.ones(P2, np.float32)
    wcol[2 * np.array(MARK)] = MW
    wcol[2 * np.array(MARK) + 1] = MW
    best2 = np.ascontiguousarray(np.asarray(best, dtype=np.float32).reshape(B, P2) * wcol)
    bgt2 = np.ascontiguousarray(np.asarray(best_gt, dtype=np.float32).reshape(B, P2) * wcol)
    rest = [i for i in range(P) if i not in MARK]
    perm = np.array(list(MARK) + rest)

    # combined gather rows: [rzs | pts zero-padded to PC], bf16
    comb = np.zeros((B, S, CW), dtype=BF)
    comb[:, :, :D] = np.asarray(rzs, dtype=np.float32).reshape(B, S, D).astype(BF)
    comb[:, :, D : D + P2] = (
        np.asarray(pts, dtype=np.float32)[:, :, perm, :].reshape(B, S, P2).astype(BF)
    )
    gts_p = np.zeros((B, SG, PC), dtype=BF)
    gts_p[:, :, :P2] = (
        np.asarray(gts, dtype=np.float32)[:, :, perm, :].reshape(B, SG, P2).astype(BF)
    )

    pp = np.arange(128)
    in_maps = []
    for c in range(NCORES):
        sl = slice(c * BL, (c + 1) * BL)
        mp = mapping[sl]  # [BL, SG]
        b = pp // 8
        pos = 16 * (pp % 8)[:, None] + np.arange(BL)[None, :]
        idx2 = (b[:, None] * S + mp[b[:, None], pos]).astype(np.int32)
        # zs rows reordered so partition p holds rows 16p..16p+15:
        # row 16p+k = zs[b, 16q+k] -> natural order already (b-major, i-minor)
        in_maps.append(
            {
                "zs": np.ascontiguousarray(zs[sl, :SG].reshape(BL * SG, D).astype(BF)),
                "comb": comb[sl].reshape(BL * S, CW),
                "gts": gts_p[sl].reshape(BL * SG, PC),
                "qy": qy[sl].reshape(BL * S, V),
                "best": np.ascontiguousarray(best2[sl]),
                "best_gt": np.ascontiguousarray(bgt2[sl]),
                "idx2": np.ascontiguousarray(idx2),
            }
        )

    last_results = run_bass_kernel_spmd(
        _module, in_maps, list(range(NCORES)), **trace_kwargs
    )
    parts = np.stack(
        [
            np.asarray(r["out"], dtype=np.float64).reshape(128, NSTAT).sum(axis=0)
            for r in last_results.results
        ]
    )
    tot = parts.sum(axis=0)

    ae_loss = tot[C_AE : C_AE + NAE].sum() / (B * SG * D)
    bias_sq = tot[C_BIAS : C_BIAS + NPT].sum()
    mark_sq = tot[C_MARK : C_MARK + NPT].sum()
    bias_loss = bias_sq / (B * SG * P2) + ALPHA * mark_sq / (B * SG * 2 * len(MARK))
    kld_loss = tot[C_KLD] / (B * S)
    best_mse = tot[C_BEST] / (B * P2)

    return np.array(kld_loss + ae_loss + best_mse + bias_loss, dtype=np.float32)



# revision 26
# speedup vs baseline: 1.0643x; 1.0039x over previous
"""CQVAE loss kernel for Trainium2, data-parallel over batch on 8 NeuronCores.

loss = kld(qy) + mse(gather(rzs), zs[:, :Sg]) + bias(best, best_gt)
       + bias(gather(pts), gts)
where bias(p, g) = mse(p, g) + 10 * mse(p[..., MARK, :], g[..., MARK, :]).

Each core handles 16 of the 128 batches.  The mapping-gathers run as
dma_gather ops (hundreds of rows per op, ~9ns/row of Q7 emission)
interleaved so gather bytes, zs bytes and compute pipeline smoothly.
pts/gts rows are zero-padded to 256 floats on the host so gathered rows
are 1KB-aligned and pad columns contribute nothing to the sums.  zs/gts
are laid out so every partition reads one contiguous 64/16KB run.  Each
core ships a [128, 32] per-partition stats tile; the host folds
partitions and cores.
"""

import sys

import numpy as np

try:
    import concourse  # noqa: F401
except ImportError:  # pragma: no cover
    sys.path.insert(0, "/opt/trn_rl_repo")

import ml_dtypes

import concourse.bass as bass  # noqa: F401
import concourse.mybir as mybir
import concourse.tile as tile
from concourse import bacc
from concourse.bass_utils import run_bass_kernel_spmd

F32 = mybir.dt.float32
BF16 = mybir.dt.bfloat16
I32 = mybir.dt.int32
AX = mybir.AxisListType
OP = mybir.AluOpType
ACTF = mybir.ActivationFunctionType

NCORES = 8
B, S, SG, D, P, V = 128, 256, 128, 1024, 118, 64
BL = B // NCORES  # batches per core
P2 = 2 * P  # 236 true floats per point-row
PC = 256  # padded point-row width
MARK = (0, 29, 88, 117)
EPS = 1e-20
ALPHA = 10.0

NSTAT = 36
# stats columns
C_KLD = 33
C_BEST, C_BESTM = 10, 11
C_AE = 0  # 10 cols: ae pieces
C_BIAS = 12  # 4 cols: bias sq totals per pts quarter
C_MARK = 16  # 16 cols: 4 marks x 4 quarters

CW = D + PC  # 1280 combined row width

# rzs pieces by (start_slot, n_slots): coarse early, 1-slot at the end
AE_PIECES = [(0, 2), (2, 2), (4, 2), (6, 2), (8, 2), (10, 2),
             (12, 1), (13, 1), (14, 1), (15, 1)]
NAE = len(AE_PIECES)
NPT = 4  # pts gather ops / gts quarters (4 batches each)
KP = BL // NPT  # 4 batch-slots per pts quarter

_module = None
last_results = None  # BassKernelResults of the most recent run (for profiling)


def _build_module():
    nc = bacc.Bacc()

    zs = nc.dram_tensor("zs", [BL * SG, D], BF16, kind="ExternalInput")
    # comb row r = concat(rzs[r], pts_padded[r]) — one gather fetches both
    comb = nc.dram_tensor("comb", [BL * S, CW], BF16, kind="ExternalInput")
    gts = nc.dram_tensor("gts", [BL * SG, PC], BF16, kind="ExternalInput")
    qy = nc.dram_tensor("qy", [BL * S, V], BF16, kind="ExternalInput")
    best = nc.dram_tensor("best", [BL, P2], F32, kind="ExternalInput")
    best_gt = nc.dram_tensor("best_gt", [BL, P2], F32, kind="ExternalInput")
    # idx[p, k] = (p//8)*S + mapping[p//8, 16*(p%8) + k] — the flat source
    # row for slot k of partition p, shared by the rzs and pts gathers
    idx2 = nc.dram_tensor("idx2", [128, BL], I32, kind="ExternalInput")
    out = nc.dram_tensor("out", [128, NSTAT], F32, kind="ExternalOutput")

    QCOLS = BL * S * V // 128  # 2048
    QN = BL * S // 128  # 32 qy rows per partition

    with tile.TileContext(nc) as tc:
        with tc.tile_pool(name="cst", bufs=1) as cst:
            idx_t = cst.tile([128, BL], I32)
            nc.sync.dma_start(idx_t[:], idx2[:])

            stats = cst.tile([128, NSTAT], F32)
            nc.vector.memset(stats[:], 0.0)

            # ---- gathers: one combined-row op per slot (SWDGE queue) ------
            cb = cst.tile([128, BL * CW], BF16)
            for k in range(BL):
                nc.gpsimd.indirect_dma_start(
                    out=cb[:, k * CW : (k + 1) * CW],
                    out_offset=None,
                    in_=comb[:],
                    in_offset=bass.IndirectOffsetOnAxis(
                        ap=idx_t[:, k : k + 1], axis=0
                    ),
                )
            cb3 = cb[:].rearrange("p (k c) -> p k c", c=CW)

            # ---- direct loads --------------------------------------------
            # scalar HWDGE queue: qy, best, gts quarters (3.2 MB)
            qy_t = cst.tile([128, QCOLS], BF16)
            nc.scalar.dma_start(
                qy_t[:], qy[:].rearrange("(p n) v -> p (n v)", n=QN)
            )
            bt = cst.tile([BL, P2], F32)
            nc.scalar.dma_start(bt[:], best[:])
            bgt = cst.tile([BL, P2], F32)
            nc.scalar.dma_start(bgt[:], best_gt[:])
            # partition p holds gts rows 16p..16p+15 (contiguous 16KB)
            gts_r = gts[:].rearrange("(p k) c -> p (k c)", k=BL)
            gt_h = []
            for h in range(NPT):
                g = cst.tile([128, KP * PC], BF16, tag=f"gt{h}", name=f"gt{h}")
                nc.scalar.dma_start(g[:], gts_r[:, h * KP * PC : (h + 1) * KP * PC])
                gt_h.append(g)

            # sync HWDGE queue: zs pieces (8.4 MB)
            # partition p holds zs rows 16p..16p+15 (contiguous 64KB)
            zs_r = zs[:].rearrange("(p k) d -> p (k d)", k=BL)
            zs_t = []
            for j, (s0, ns) in enumerate(AE_PIECES):
                z = cst.tile([128, ns * D], BF16, tag=f"zs{j}", name=f"zs{j}")
                nc.sync.dma_start(z[:], zs_r[:, s0 * D : (s0 + ns) * D])
                zs_t.append(z)

            # ---- compute --------------------------------------------------
            # BEST (tiny, lands early on the scalar queue)
            nc.vector.tensor_sub(bt[:], bt[:], bgt[:])
            nc.vector.tensor_mul(bt[:], bt[:], bt[:])
            nc.vector.reduce_sum(out=stats[:BL, C_BEST : C_BEST + 1], in_=bt[:], axis=AX.X)
            bm4 = cst.tile([BL, 4], F32)
            for j, m in enumerate(MARK):
                nc.vector.reduce_sum(
                    out=bm4[:, j : j + 1], in_=bt[:, 2 * m : 2 * m + 2], axis=AX.X
                )
            nc.vector.reduce_sum(out=stats[:BL, C_BESTM : C_BESTM + 1], in_=bm4[:], axis=AX.X)

            # KLD: sum q * (log(q + eps) - log(1/V)) via log(V*q + V*eps)
            lg = cst.tile([128, QCOLS], F32)
            ebias = cst.tile([128, 1], F32)
            nc.vector.memset(ebias[:], float(V) * EPS)
            nc.scalar.activation(lg[:], qy_t[:], ACTF.Ln, bias=ebias[:], scale=float(V))
            nc.vector.scalar_tensor_tensor(
                out=lg[:],
                in0=lg[:],
                scalar=0.0,
                in1=qy_t[:],
                op0=OP.subtract,
                op1=OP.mult,
                accum_out=stats[:, C_KLD : C_KLD + 1],
            )

            def ae_piece(j, on_dve=False):
                s0, ns = AE_PIECES[j]
                rg = cb3[:, s0 : s0 + ns, 0:D]
                z3 = zs_t[j][:].rearrange("p (k d) -> p k d", d=D)
                nc.vector.tensor_sub(rg, rg, z3)
                acc = stats[:, C_AE + j : C_AE + j + 1]
                if on_dve:
                    # square-and-accumulate on DVE; the consumed zs tile is
                    # the scratch destination (no operand aliasing)
                    nc.vector.scalar_tensor_tensor(
                        out=z3, in0=rg, scalar=0.0, in1=rg,
                        op0=OP.subtract, op1=OP.mult, accum_out=acc,
                    )
                else:
                    nc.scalar.activation(rg, rg, ACTF.Square, accum_out=acc)

            def bias_quarter(h, on_dve=False):
                pg = cb3[:, h * KP : (h + 1) * KP, D : D + PC]
                g3 = gt_h[h][:].rearrange("p (k c) -> p k c", c=PC)
                nc.vector.tensor_sub(pg, pg, g3)
                acc = stats[:, C_BIAS + h : C_BIAS + h + 1]
                if on_dve:
                    # squares land in the consumed gts tile; marks read there
                    nc.vector.scalar_tensor_tensor(
                        out=g3, in0=pg, scalar=0.0, in1=pg,
                        op0=OP.subtract, op1=OP.mult, accum_out=acc,
                    )
                    sq = g3
                else:
                    nc.scalar.activation(pg, pg, ACTF.Square, accum_out=acc)
                    sq = pg
                cm = C_MARK + 4 * h
                for j, m in enumerate(MARK):
                    nc.vector.reduce_sum(
                        out=stats[:, cm + j : cm + j + 1],
                        in_=sq[:, :, 2 * m : 2 * m + 2],
                        axis=AX.XY,
                    )

            # compute in data-arrival order
            ae_piece(0)
            ae_piece(1)
            bias_quarter(0)
            ae_piece(2)
            bias_quarter(1)
            ae_piece(3)
            bias_quarter(2)
            ae_piece(4)
            bias_quarter(3, on_dve=True)
            ae_piece(5)
            ae_piece(6, on_dve=True)
            ae_piece(7)
            ae_piece(8, on_dve=True)
            ae_piece(9)

            nc.sync.dma_start(out[:], stats[:])

    nc.compile()
    return nc


def kernel(
    zs, rzs, pts, best, qy, gts, best_gt, mapping, vector_dims, **trace_kwargs
):
    global _module, last_results
    vd = int(np.asarray(vector_dims))
    assert vd == V, f"kernel compiled for vector_dims={V}, got {vd}"

    if _module is None:
        _module = _build_module()

    BF = ml_dtypes.bfloat16
    zs = np.asarray(zs, dtype=np.float32)
    qy = np.asarray(qy, dtype=np.float32).astype(BF)
    mapping = np.asarray(mapping).astype(np.int32)
    best2 = np.ascontiguousarray(np.asarray(best, dtype=np.float32).reshape(B, P2))
    bgt2 = np.ascontiguousarray(np.asarray(best_gt, dtype=np.float32).reshape(B, P2))

    # combined gather rows: [rzs | pts zero-padded to PC], bf16
    comb = np.zeros((B, S, CW), dtype=BF)
    comb[:, :, :D] = np.asarray(rzs, dtype=np.float32).reshape(B, S, D).astype(BF)
    comb[:, :, D : D + P2] = (
        np.asarray(pts, dtype=np.float32).reshape(B, S, P2).astype(BF)
    )
    gts_p = np.zeros((B, SG, PC), dtype=BF)
    gts_p[:, :, :P2] = np.asarray(gts, dtype=np.float32).reshape(B, SG, P2).astype(BF)

    pp = np.arange(128)
    in_maps = []
    for c in range(NCORES):
        sl = slice(c * BL, (c + 1) * BL)
        mp = mapping[sl]  # [BL, SG]
        b = pp // 8
        pos = 16 * (pp % 8)[:, None] + np.arange(BL)[None, :]
        idx2 = (b[:, None] * S + mp[b[:, None], pos]).astype(np.int32)
        # zs rows reordered so partition p holds rows 16p..16p+15:
        # row 16p+k = zs[b, 16q+k] -> natural order already (b-major, i-minor)
        in_maps.append(
            {
                "zs": np.ascontiguousarray(zs[sl, :SG].reshape(BL * SG, D).astype(BF)),
                "comb": comb[sl].reshape(BL * S, CW),
                "gts": gts_p[sl].reshape(BL * SG, PC),
                "qy": qy[sl].reshape(BL * S, V),
                "best": np.ascontiguousarray(best2[sl]),
                "best_gt": np.ascontiguousarray(bgt2[sl]),
                "idx2": np.ascontiguousarray(idx2),
            }
        )

    last_results = run_bass_kernel_spmd(
        _module, in_maps, list(range(NCORES)), **trace_kwargs
    )
    parts = np.stack(
        [
            np.asarray(r["out"], dtype=np.float64).reshape(128, NSTAT).sum(axis=0)
            for r in last_results.results
        ]
    )
    tot = parts.sum(axis=0)

    ae_loss = tot[C_AE : C_AE + NAE].sum() / (B * SG * D)
    bias_sq = tot[C_BIAS : C_BIAS + NPT].sum()
    mark_sq = tot[C_MARK : C_MARK + 4 * NPT].sum()
    bias_loss = bias_sq / (B * SG * P2) + ALPHA * mark_sq / (B * SG * 2 * len(MARK))
    kld_loss = tot[C_KLD] / (B * S)
    best_mse = tot[C_BEST] / (B * P2) + ALPHA * tot[C_BESTM] / (B * 2 * len(MARK))

    return np.array(kld_loss + ae_loss + best_mse + bias_loss, dtype=np.float32)

